# revision 9
# baseline (speedup 1.0000x reference)
"""MoE expert-parallel kernel for Trainium2 (Bass/Tile).

8 experts, 8 NeuronCores, one expert per core (SPMD, no collectives).
Per core: out = gelu(x @ w1) @ w2 with
  x  [2048, 1024] f32, w1 [1024, 4096] f32, w2 [4096, 1024] f32.

Structure per core:
  Phase 0: DMA x row-tiles, PE-transpose to xT [k=128p, 8, 2048] in SBUF.
  For each n-block (256 wide) of the intermediate dim:
    - stream w1[:, blk] and w2[blk, :] from HBM (each weight read once total)
    - GEMM1 (float32r, free dim 512): psum[n128, tok512] over 8 k-tiles
    - GELU eviction ACT: psum -> hT_blk [n128p, 2, 2048] SBUF
    - GEMM2 (float32r): psum[tok128, d512] over the block's 2 n-tiles,
      DVE-accumulated into out_acc [tok128p, 16, 1024] SBUF
  Last block: DMA out_acc row-tiles to HBM.
"""

import os
import sys

import numpy as np

if os.path.isdir("/opt/trn_rl_repo") and "/opt/trn_rl_repo" not in sys.path:
    sys.path.insert(0, "/opt/trn_rl_repo")

# ---------------------------------------------------------------------------
# Workaround for walrus codegen "Too many sync wait commands": this compiler
# build encodes at most 2 sem-waits per engine instruction (1 for ctrl-type
# ops lowered to the TPB CTRL "NO" struct). Split excess on_wait entries onto
# NoOp instructions inserted immediately before the offender on the same
# engine — identical semantics, since an engine executes its stream in order.
_CTRL_OPCODES = {"NoOp", "Drain", "EventSemaphore", "AllEngineBarrier", "Halt",
                 "UnconditionalBranch", "BranchHint"}


def _split_excess_waits(bir_json: bytes) -> bytes:
    import orjson

    d = orjson.loads(bir_json)
    changed = False
    for fn in d.get("functions", []):
        for blk in fn.get("blocks", []):
            out = []
            for ins in blk.get("instructions", []):
                sync = ins.get("sync_info")
                waits = (sync or {}).get("on_wait") or []
                cap = 1
                if len(waits) > cap:
                    changed = True
                    extra, keep = waits[:-cap], waits[-cap:]
                    for i in range(0, len(extra), 1):
                        out.append({
                            "name": f"{ins['name']}-wsplit{i}",
                            "opcode": "NoOp",
                            "engine": ins["engine"],
                            "ins": [],
                            "outs": [],
                            "debug": ins.get("debug", 0),
                            "sync_info": {"on_update": [], "on_wait": [extra[i]]},
                        })
                    sync["on_wait"] = keep
                out.append(ins)
            blk["instructions"] = out
    return orjson.dumps(d) if changed else bir_json


_hook_installed = False


def _install_wait_split_hook():
    global _hook_installed
    if _hook_installed:
        return
    import concourse.bass2jax as bass2jax
    import concourse.bass_utils as bass_utils

    orig = bass_utils.compile_bir_kernel

    def patched(bir_json, tmpdir, neff_name="file.neff"):
        return orig(_split_excess_waits(bir_json), tmpdir, neff_name)

    bass2jax.compile_bir_kernel = patched
    bass_utils.compile_bir_kernel = patched
    _hook_installed = True


NUM_EXPERTS = 8
TOK = 2048
HID = 1024
INT = 4096
OUT = 1024
P = 128

NB = 256             # n-block width (intermediate dim)
NBLK = INT // NB     # 16
NT = NB // P         # 2 n-tiles per block
TCH = 512            # tok chunk (GEMM1 moving free dim)
NCH = TOK // TCH     # 4
KT = HID // P        # 8 k-tiles for GEMM1
TT = TOK // P        # 16 tok tiles
ND = OUT // 512      # 2 d-halves for GEMM2 moving dim

_PROGRAM_CACHE = {}


def build_program(mm_dtype_name=None):
    """Build the per-core Bass program. Returns the finalized Bass object."""
    import concourse.bass as bass
    import concourse.tile as tile
    from concourse import mybir
    from concourse.masks import make_identity

    f32 = mybir.dt.float32
    if mm_dtype_name is None:
        mm_dtype_name = os.environ.get("MOE_MM_DTYPE", "float32r")
    mm_dt = getattr(mybir.dt, mm_dtype_name)

    # Tiles feeding f32r matmuls are allocated as float32r and written by
    # their producers at that dtype (the BIR verifier requires matmul
    # operands to be pre-rounded to FP32r; ACT rounds on write, DMA loads
    # reinterpret the f32 bits via a bitcast of the DRAM-side AP).
    def dma_src(ap):
        if mm_dt is f32:
            return ap
        return ap.bitcast(mm_dt)

    nc = bass.Bass()
    x_h = nc.declare_dram_parameter("x", [TOK, HID], f32, isOutput=False)
    w1_h = nc.declare_dram_parameter("w1", [HID, INT], f32, isOutput=False)
    w2_h = nc.declare_dram_parameter("w2", [INT, OUT], f32, isOutput=False)
    out_h = nc.declare_dram_parameter("out", [TOK, OUT], f32, isOutput=True)

    x_ap = x_h[:, :]
    out_ap = out_h[:, :]
    # w1 [k, n] -> [p, a, n] with k = a*128 + p  (partition = k within tile)
    w1r = w1_h[:, :].rearrange("(a p) n -> p a n", p=P)
    # w2 [n, d] -> [p, a, d] with n = a*128 + p
    w2r = w2_h[:, :].rearrange("(a p) d -> p a d", p=P)

    gelu = getattr(
        mybir.ActivationFunctionType, os.environ.get("MOE_ACT", "Gelu")
    )

    with tile.TileContext(nc) as tc:
        with (
            tc.tile_pool(name="const", bufs=1) as const_pool,
            tc.tile_pool(name="xrow", bufs=3) as xrow_pool,
            tc.tile_pool(name="tpsum", bufs=2, space="PSUM") as tpsum_pool,
            tc.tile_pool(name="xtp", bufs=1) as xt_pool,
            tc.tile_pool(name="w1p", bufs=2) as w1_pool,
            tc.tile_pool(name="w2p", bufs=2) as w2_pool,
            tc.tile_pool(name="htp", bufs=1) as ht_pool,
            tc.tile_pool(name="hpsum", bufs=3, space="PSUM") as hpsum_pool,
            tc.tile_pool(name="opsum", bufs=3, space="PSUM") as opsum_pool,
            tc.tile_pool(name="outp", bufs=1) as out_pool,
        ):
            ident = const_pool.tile([P, P], f32, name="ident")
            make_identity(nc, ident)

            xt = xt_pool.tile([P, KT, TOK], mm_dt, name="xt")
            out_acc = out_pool.tile([P, TT, OUT], f32, name="out_acc")

            # ---- Phase 0: transpose x into xt --------------------------------
            for r in range(TT):
                xrow = xrow_pool.tile([P, HID], f32, name="xrow")
                nc.sync.dma_start(out=xrow[:], in_=x_ap[r * P:(r + 1) * P, :])
                for k in range(KT):
                    tp = tpsum_pool.tile([P, P], f32, name="tp")
                    nc.tensor.transpose(tp[:], xrow[:, k * P:(k + 1) * P], ident[:])
                    nc.scalar.copy(xt[:, k, r * P:(r + 1) * P], tp[:])

            # ---- Main loop over n-blocks ------------------------------------
            for b in range(NBLK):
                w1b = w1_pool.tile([P, KT, NB], mm_dt, name="w1b")
                nc.sync.dma_start(out=w1b[:], in_=dma_src(w1r[:, :, b * NB:(b + 1) * NB]))
                w2b = w2_pool.tile([P, NT, OUT], mm_dt, name="w2b")
                nc.sync.dma_start(out=w2b[:], in_=dma_src(w2r[:, b * NT:(b + 1) * NT, :]))

                htb = ht_pool.tile([P, NT, TOK], mm_dt, name="htb")

                # GEMM1: hT[n, tok] = w1[:, n].T @ xT ; GELU into htb
                for j in range(NT):
                    for c in range(NCH):
                        ph = hpsum_pool.tile([P, TCH], f32, name="ph")
                        for k in range(KT):
                            nc.tensor.matmul(
                                ph[:],
                                w1b[:, k, j * P:(j + 1) * P],
                                xt[:, k, c * TCH:(c + 1) * TCH],
                                start=(k == 0),
                                stop=(k == KT - 1),
                            )
                        nc.scalar.activation(
                            htb[:, j, c * TCH:(c + 1) * TCH], ph[:], gelu
                        )

                # GEMM2: out[tok, d] += hT[:, tok].T @ w2[blk, d]
                for t in range(TT):
                    for d in range(ND):
                        po = opsum_pool.tile([P, 512], f32, name="po")
                        for j in range(NT):
                            nc.tensor.matmul(
                                po[:],
                                htb[:, j, t * P:(t + 1) * P],
                                w2b[:, j, d * 512:(d + 1) * 512],
                                start=(j == 0),
                                stop=(j == NT - 1),
                            )
                        if b == 0:
                            nc.vector.tensor_copy(
                                out_acc[:, t, d * 512:(d + 1) * 512], po[:]
                            )
                        else:
                            nc.vector.tensor_add(
                                out_acc[:, t, d * 512:(d + 1) * 512],
                                out_acc[:, t, d * 512:(d + 1) * 512],
                                po[:],
                            )
                    if b == NBLK - 1:
                        nc.sync.dma_start(
                            out=out_ap[t * P:(t + 1) * P, :], in_=out_acc[:, t, :]
                        )

    return nc


def _get_program():
    key = os.environ.get("MOE_MM_DTYPE", "float32r")
    if key not in _PROGRAM_CACHE:
        _PROGRAM_CACHE[key] = build_program(key)
    return _PROGRAM_CACHE[key]


def kernel(x, w1, w2, _trace=False, _trace_kwargs=None):
    """Full-tensor entry point: shards experts across 8 cores, returns full out."""
    from concourse.bass_utils import run_bass_kernel_spmd

    _install_wait_split_hook()
    x = np.ascontiguousarray(x, dtype=np.float32)
    w1 = np.ascontiguousarray(w1, dtype=np.float32)
    w2 = np.ascontiguousarray(w2, dtype=np.float32)
    assert x.shape == (NUM_EXPERTS, TOK, HID)
    assert w1.shape == (NUM_EXPERTS, HID, INT)
    assert w2.shape == (NUM_EXPERTS, INT, OUT)

    nc = _get_program()
    core_ids = list(range(NUM_EXPERTS))
    in_maps = [
        {"x": x[e], "w1": w1[e], "w2": w2[e]} for e in range(NUM_EXPERTS)
    ]
    kw = {}
    if _trace:
        kw["trace"] = True
        kw["trace_kwargs"] = _trace_kwargs or {}
    res = run_bass_kernel_spmd(nc, in_maps, core_ids, **kw)
    out = np.stack([res.results[e]["out"] for e in range(NUM_EXPERTS)], axis=0)
    if _trace:
        return out, res
    return out


if __name__ == "__main__":
    rng = np.random.default_rng(0)
    x = rng.standard_normal((NUM_EXPERTS, TOK, HID), dtype=np.float32)
    w1 = rng.standard_normal((NUM_EXPERTS, HID, INT), dtype=np.float32) * 0.03
    w2 = rng.standard_normal((NUM_EXPERTS, INT, OUT), dtype=np.float32) * 0.015
    out = kernel(x, w1, w2)
    print("out", out.shape, out.dtype, float(np.abs(out).mean()))


# revision 13
# speedup vs baseline: 105.6095x; 105.6095x over previous
"""MoE expert-parallel kernel for Trainium2 (Bass/Tile).

8 experts, 8 NeuronCores, one expert per core (SPMD, no collectives).
Per core: out = gelu(x @ w1) @ w2 with
  x  [2048, 1024] f32, w1 [1024, 4096] f32, w2 [4096, 1024] f32.

Matmuls run as float32r (TF32-class PE fast path, fp32 storage): rel err vs
the fp32 reference ~2e-4.

Structure per core — two token-halves of 1024, each:
  Phase 0: DMA x row-tiles, PE-transpose to xT [k=128p, 8, 1024] in SBUF.
  For each of 8 n-blocks (512 wide) of the intermediate dim:
    - stream w1[:, blk] and w2[blk, :] from HBM
    - GEMM1 (f32r, free dim 512): psum[n128, tok512] over 8 k-tiles
    - GELU eviction on ACT: psum -> hT_blk [n128p, 4, 1024] SBUF (rounds f32r)
    - GEMM2 (f32r): psum[tok128, d512] over the block's 4 n-tiles,
      DVE-accumulated into out_acc [tok128p, 8, 1024] f32 SBUF
  After the half's last block: DMA out row-tiles to HBM.
"""

import os
import sys

import numpy as np

if os.path.isdir("/opt/trn_rl_repo") and "/opt/trn_rl_repo" not in sys.path:
    sys.path.insert(0, "/opt/trn_rl_repo")

# ---------------------------------------------------------------------------
# Workaround for walrus codegen "Too many sync wait commands": this compiler
# build encodes at most 1 sem-wait per instruction. Split excess on_wait
# entries onto NoOp instructions inserted immediately before the offender on
# the same engine — identical semantics, since an engine executes its stream
# in order.


def _split_excess_waits(bir_json: bytes) -> bytes:
    import orjson

    d = orjson.loads(bir_json)
    changed = False
    for fn in d.get("functions", []):
        for blk in fn.get("blocks", []):
            out = []
            for ins in blk.get("instructions", []):
                sync = ins.get("sync_info")
                waits = (sync or {}).get("on_wait") or []
                cap = 1
                if len(waits) > cap:
                    changed = True
                    extra, keep = waits[:-cap], waits[-cap:]
                    for i in range(len(extra)):
                        out.append({
                            "name": f"{ins['name']}-wsplit{i}",
                            "opcode": "NoOp",
                            "engine": ins["engine"],
                            "ins": [],
                            "outs": [],
                            "debug": ins.get("debug", 0),
                            "sync_info": {"on_update": [], "on_wait": [extra[i]]},
                        })
                    sync["on_wait"] = keep
                out.append(ins)
            blk["instructions"] = out
    return orjson.dumps(d) if changed else bir_json


_hook_installed = False


def _install_wait_split_hook():
    global _hook_installed
    if _hook_installed:
        return
    import concourse.bass2jax as bass2jax
    import concourse.bass_utils as bass_utils

    orig = bass_utils.compile_bir_kernel

    def patched(bir_json, tmpdir, neff_name="file.neff"):
        return orig(_split_excess_waits(bir_json), tmpdir, neff_name)

    bass2jax.compile_bir_kernel = patched
    bass_utils.compile_bir_kernel = patched
    _hook_installed = True


NUM_EXPERTS = 8
TOK = 2048
HID = 1024
INT = 4096
OUT = 1024
P = 128

TH = 1024            # token half
NTH = TOK // TH      # 2 halves
NB = 512             # n-block width (intermediate dim)
NBLK = INT // NB     # 8 blocks
NT = NB // P         # 4 n-tiles per block
TCH = 512            # tok chunk (GEMM1 moving free dim)
NCH = TH // TCH      # 2 chunks per half
KT = HID // P        # 8 k-tiles for GEMM1
TT = TH // P         # 8 tok tiles per half
ND = OUT // 512      # 2 d-halves for GEMM2 moving dim

_PROGRAM_CACHE = {}


def build_program(mm_dtype_name=None, repeats=1):
    """Build the per-core Bass program. Returns the finalized Bass object.

    repeats>1 re-emits the whole kernel body that many times in one program
    (used only for timing-by-amplification in test.py).
    """
    import concourse.bass as bass
    import concourse.tile as tile
    from concourse import mybir
    from concourse.masks import make_identity

    f32 = mybir.dt.float32
    if mm_dtype_name is None:
        mm_dtype_name = os.environ.get("MOE_MM_DTYPE", "float32r")
    if mm_dtype_name == "mixed":
        # GEMM1 in f32r (precision), GEMM2 in bf16 (speed)
        g1_dt = mybir.dt.float32r
        g2_dt = mybir.dt.bfloat16
    else:
        g1_dt = g2_dt = getattr(mybir.dt, mm_dtype_name)

    nc = bass.Bass()
    x_h = nc.declare_dram_parameter("x", [TOK, HID], f32, isOutput=False)
    w1_h = nc.declare_dram_parameter("w1", [HID, INT], f32, isOutput=False)
    w2_h = nc.declare_dram_parameter("w2", [INT, OUT], f32, isOutput=False)
    out_h = nc.declare_dram_parameter("out", [TOK, OUT], f32, isOutput=True)

    x_ap = x_h[:, :]
    out_ap = out_h[:, :]
    # w1 [k, n] -> [p, a, n] with k = a*128 + p  (partition = k within tile)
    w1r = w1_h[:, :].rearrange("(a p) n -> p a n", p=P)
    # w2 [n, d] -> [p, a, d] with n = a*128 + p
    w2r = w2_h[:, :].rearrange("(a p) d -> p a d", p=P)

    gelu = getattr(
        mybir.ActivationFunctionType, os.environ.get("MOE_ACT", "Gelu")
    )

    with tile.TileContext(nc) as tc:
        with (
            tc.tile_pool(name="const", bufs=1) as const_pool,
            tc.tile_pool(name="xrow", bufs=2) as xrow_pool,
            tc.tile_pool(name="tpsum", bufs=2, space="PSUM") as tpsum_pool,
            tc.tile_pool(name="xtp", bufs=1) as xt_pool,
            tc.tile_pool(name="w1p", bufs=2) as w1_pool,
            tc.tile_pool(name="w2p", bufs=2) as w2_pool,
            tc.tile_pool(name="htp", bufs=2) as ht_pool,
            tc.tile_pool(name="hpsum", bufs=3, space="PSUM") as hpsum_pool,
            tc.tile_pool(name="opsum", bufs=3, space="PSUM") as opsum_pool,
            tc.tile_pool(name="outp", bufs=1) as out_pool,
        ):
            ident = const_pool.tile([P, P], f32, name="ident")
            make_identity(nc, ident)

            pools = (xt_pool, out_pool, xrow_pool, tpsum_pool, w1_pool,
                     w2_pool, ht_pool, hpsum_pool, opsum_pool)
            for _rep in range(repeats):
                for half in range(NTH):
                    _emit_half(nc, tc, half, g1_dt, g2_dt, f32, gelu,
                               x_ap, out_ap, w1r, w2r, pools, ident)

    return nc


def _emit_half(nc, tc, half, g1_dt, g2_dt, f32, gelu, x_ap, out_ap,
               w1r, w2r, pools, ident):
    (xt_pool, out_pool, xrow_pool, tpsum_pool, w1_pool, w2_pool, ht_pool,
     hpsum_pool, opsum_pool) = pools
    from concourse import mybir
    t0 = half * TH  # first token of this half

    def wdma(out_tile, in_ap_f32, wdt):
        # weight load: HWDGE bitcast for 4-byte matmul dtypes (same bits),
        # SWDGE casting DMA for 2-byte dtypes (gpsimd converts f32->bf16)
        if mybir.dt.size(wdt) == 2:
            nc.gpsimd.dma_start(out=out_tile, in_=in_ap_f32)
        elif wdt is f32:
            nc.sync.dma_start(out=out_tile, in_=in_ap_f32)
        else:
            nc.sync.dma_start(out=out_tile, in_=in_ap_f32.bitcast(wdt))

    xt = xt_pool.tile([P, KT, TH], g1_dt, name="xt")
    out_acc = out_pool.tile([P, TT, OUT], f32, name="out_acc")

    def transpose_chunk(c):
        # transpose the 4 x row-tiles backing tok chunk c into xt
        for r in range(4 * c, 4 * (c + 1)):
            xrow = xrow_pool.tile([P, HID], f32, name="xrow")
            nc.sync.dma_start(
                out=xrow[:], in_=x_ap[t0 + r * P:t0 + (r + 1) * P, :]
            )
            for k in range(KT):
                tp = tpsum_pool.tile([P, P], f32, name="tp")
                nc.tensor.transpose(
                    tp[:], xrow[:, k * P:(k + 1) * P], ident[:]
                )
                nc.scalar.copy(xt[:, k, r * P:(r + 1) * P], tp[:])

    # ---- Main loop over n-blocks --------------------------------------
    for b in range(NBLK):
        w1b = w1_pool.tile([P, KT, NB], g1_dt, name="w1b")
        w2b = w2_pool.tile([P, NT, OUT], g2_dt, name="w2b")
        for j in range(NT):
            # split weight DMAs by n-tile so GEMM1 group j can start as
            # soon as its own slice has landed
            wdma(w1b[:, :, j * P:(j + 1) * P],
                 w1r[:, :, b * NB + j * P:b * NB + (j + 1) * P], g1_dt)
            wdma(w2b[:, j, :], w2r[:, b * NT + j, :], g2_dt)

        htb = ht_pool.tile([P, NT, TH], g2_dt, name="htb")

        # GEMM1: hT[n, tok] = w1[:, n].T @ xT ; GELU into htb
        # chunk-major so the half's first transposes overlap block 0's GEMM1
        for c in range(NCH):
            if b == 0:
                transpose_chunk(c)
            for j in range(NT):
                ph = hpsum_pool.tile([P, TCH], f32, name="ph")
                for k in range(KT):
                    nc.tensor.matmul(
                        ph[:],
                        w1b[:, k, j * P:(j + 1) * P],
                        xt[:, k, c * TCH:(c + 1) * TCH],
                        start=(k == 0),
                        stop=(k == KT - 1),
                    )
                nc.scalar.activation(
                    htb[:, j, c * TCH:(c + 1) * TCH], ph[:], gelu
                )

        # GEMM2: out[tok, d] += hT[:, tok].T @ w2[blk, d]
        for t in range(TT):
            for d in range(ND):
                po = opsum_pool.tile([P, 512], f32, name="po")
                for j in range(NT):
                    nc.tensor.matmul(
                        po[:],
                        htb[:, j, t * P:(t + 1) * P],
                        w2b[:, j, d * 512:(d + 1) * 512],
                        start=(j == 0),
                        stop=(j == NT - 1),
                    )
                if b == 0:
                    nc.vector.tensor_copy(
                        out_acc[:, t, d * 512:(d + 1) * 512], po[:]
                    )
                else:
                    nc.vector.tensor_add(
                        out_acc[:, t, d * 512:(d + 1) * 512],
                        out_acc[:, t, d * 512:(d + 1) * 512],
                        po[:],
                    )
            if b == NBLK - 1:
                nc.sync.dma_start(
                    out=out_ap[t0 + t * P:t0 + (t + 1) * P, :],
                    in_=out_acc[:, t, :],
                )


def _get_program():
    key = os.environ.get("MOE_MM_DTYPE", "float32r")
    if key not in _PROGRAM_CACHE:
        _PROGRAM_CACHE[key] = build_program(key)
    return _PROGRAM_CACHE[key]


def kernel(x, w1, w2, _trace=False, _trace_kwargs=None):
    """Full-tensor entry point: shards experts across 8 cores, returns full out."""
    from concourse.bass_utils import run_bass_kernel_spmd

    _install_wait_split_hook()
    x = np.ascontiguousarray(x, dtype=np.float32)
    w1 = np.ascontiguousarray(w1, dtype=np.float32)
    w2 = np.ascontiguousarray(w2, dtype=np.float32)
    assert x.shape == (NUM_EXPERTS, TOK, HID)
    assert w1.shape == (NUM_EXPERTS, HID, INT)
    assert w2.shape == (NUM_EXPERTS, INT, OUT)

    nc = _get_program()
    core_ids = list(range(NUM_EXPERTS))
    in_maps = [
        {"x": x[e], "w1": w1[e], "w2": w2[e]} for e in range(NUM_EXPERTS)
    ]
    kw = {}
    if _trace:
        kw["trace"] = True
        kw["trace_kwargs"] = _trace_kwargs or {}
    res = run_bass_kernel_spmd(nc, in_maps, core_ids, **kw)
    out = np.stack([res.results[e]["out"] for e in range(NUM_EXPERTS)], axis=0)
    if _trace:
        return out, res
    return out


if __name__ == "__main__":
    rng = np.random.default_rng(0)
    x = rng.standard_normal((NUM_EXPERTS, TOK, HID), dtype=np.float32)
    w1 = rng.standard_normal((NUM_EXPERTS, HID, INT), dtype=np.float32) * 0.03
    w2 = rng.standard_normal((NUM_EXPERTS, INT, OUT), dtype=np.float32) * 0.015
    out = kernel(x, w1, w2)
    print("out", out.shape, out.dtype, float(np.abs(out).mean()))
